# revision 16
# baseline (speedup 1.0000x reference)
"""Bass/Trainium2 kernel for a 2-layer bidirectional masked LSTM encoder.

Problem shapes (hardcoded): B=64, T=256, INSZ=512, HSZ=1024, H=512.
8 NeuronCores, SPMD single program:
  - input GEMMs (xw = x @ W + b) are sharded across cores by GATE COLUMNS:
    core j receives only its 256-column slice of W as input data, so the
    program is identical on every core; chunked AllGathers (t-blocks of 16)
    assemble the full xw on every core;
  - the sequential LSTM recurrence is replicated on every core at full
    batch (it is N-streaming bound on the PE, so batch width is free):
    fwd direction on partitions 0-63, bwd on partitions 64-127; col-tiled
    matmul pairs (tile_position (0,0)/(0,64), auto-derived) run concurrently;
  - xw_t is folded into the recurrent matmul's accumulation group through
    an identity-block matmul (start=False), avoiding DVE adds;
  - masking (Keras carry-over semantics) via copy_predicated with a
    per-partition mask column; the bwd half of the mask is time-reversed
    on the host so one fused op serves both directions.
Precision: weights/gates/h in fp16, c-state and PSUM accumulation fp32.
"""

import os
import sys

sys.path.insert(0, "/opt/trn_rl_repo")

import numpy as np
from contextlib import ExitStack

import concourse.bass as bass
import concourse.tile as tile
from concourse import bacc, mybir
from concourse._compat import with_exitstack

F32 = mybir.dt.float32
F16 = mybir.dt.float16
AF = mybir.ActivationFunctionType

NCORES = 8
B = 64             # full batch
T = 256
INSZ = 512
H = 512            # per-direction hidden
G = 4 * H          # gate width (2048)
GSL = G // NCORES  # per-core gate-column slice (256)
TCH = 16           # t-block chunk size for allgathers
NCH = T // TCH     # number of chunks (16)
MT_CH = TCH * B // 128   # m-tiles per chunk in the GEMMs (8)

NK0 = INSZ // 128  # 4
NKH = H // 128     # 4
NK1 = 2 * H // 128  # 8 (layer-1 input dim k-tiles)


def _chunk_order(nch):
    """0, nch-1, 1, nch-2, ... so fwd (t asc) and bwd (t desc) are served."""
    order = []
    for i in range((nch + 1) // 2):
        order.append(i)
        j = nch - 1 - i
        if j != i:
            order.append(j)
    return order


@with_exitstack
def _recurrence(ctx: ExitStack, tc, nc, layer, u_sb, xw_g, mask_sb, ident,
                h0T_d, rnnout, h1_out, c1_out):
    spool = ctx.enter_context(tc.tile_pool(name=f"st{layer}", bufs=1))
    gpool = ctx.enter_context(tc.tile_pool(name=f"ga{layer}", bufs=2))
    tpool = ctx.enter_context(tc.tile_pool(name=f"tm{layer}", bufs=2))
    xpool = ctx.enter_context(tc.tile_pool(name=f"xw{layer}", bufs=3))
    zpool = ctx.enter_context(tc.tile_pool(name=f"zp{layer}", bufs=1,
                                           space="PSUM"))
    trpool = ctx.enter_context(tc.tile_pool(name=f"tr{layer}", bufs=4,
                                            space="PSUM"))

    c_st = spool.tile([128, H], F32)
    h_st = spool.tile([128, H], F16)
    hT = spool.tile([128, NKH, 128], F16)
    nc.gpsimd.memset(c_st[:], 0.0)
    nc.gpsimd.memset(h_st[:], 0.0)
    nc.gpsimd.memset(hT[:], 0.0)

    for s in range(T):
        tb = s               # fwd time index
        tr = T - 1 - s       # bwd time index

        xwf = xpool.tile([B, G], F16, tag="xwf")
        nc.gpsimd.dma_start(
            xwf[:].rearrange("b (cc n) -> b cc n", n=GSL),
            xw_g[tb // TCH, :, 0, tb % TCH, :, :].rearrange("cc b n -> b cc n"))
        xwb = xpool.tile([B, G], F16, tag="xwb")
        nc.gpsimd.dma_start(
            xwb[:].rearrange("b (cc n) -> b cc n", n=GSL),
            xw_g[tr // TCH, :, 1, tr % TCH, :, :].rearrange("cc b n -> b cc n"))

        z = zpool.tile([128, G], F32, tag="z")
        for n in range(4):
            # one accumulation group per psum bank covering both directions;
            # f/b matmuls adjacent so their col-tiled streams overlap on PE
            ns = slice(n * 512, (n + 1) * 512)
            for k in range(NKH):
                nc.tensor.matmul(z[0:64, ns], hT[:, k, 0:64],
                                 u_sb[:, 0, k, ns],
                                 start=(k == 0), stop=False)
                nc.tensor.matmul(z[64:128, ns], hT[:, k, 64:128],
                                 u_sb[:, 1, k, ns],
                                 start=(k == 0), stop=False,
                                 skip_group_check=True)
            nc.tensor.matmul(z[0:64, ns], ident[0:64, 0:64], xwf[:, ns],
                             start=False, stop=True)
            nc.tensor.matmul(z[64:128, ns], ident[0:64, 0:64], xwb[:, ns],
                             start=False, stop=True, skip_group_check=True)

        # gate columns: [i (0:512) | f (512:1024) | g (1024:1536) | o (1536:2048)]
        gates = gpool.tile([128, G], F16, tag="gates")
        nc.scalar.activation(gates[:, 0:1024], z[:, 0:1024], AF.Sigmoid)
        nc.scalar.activation(gates[:, 1024:1536], z[:, 1024:1536], AF.Tanh)
        nc.scalar.activation(gates[:, 1536:2048], z[:, 1536:2048], AF.Sigmoid)
        gi = gates[:, 0:512]
        gf = gates[:, 512:1024]
        gg = gates[:, 1024:1536]
        go = gates[:, 1536:2048]

        mb = mask_sb[:, s:s + 1].to_broadcast((128, H))

        p = tpool.tile([128, H], F16, tag="p")
        nc.vector.tensor_mul(p[:], gi, gg)
        cn1 = tpool.tile([128, H], F32, tag="cn1")
        nc.vector.tensor_mul(cn1[:], gf, c_st[:])
        cnew = tpool.tile([128, H], F32, tag="cnew")
        nc.vector.tensor_add(cnew[:], cn1[:], p[:])
        nc.vector.copy_predicated(c_st[:], mb, cnew[:])

        th = tpool.tile([128, H], F16, tag="th")
        nc.scalar.activation(th[:], c_st[:], AF.Tanh)
        ho = tpool.tile([128, H], F16, tag="ho")
        nc.vector.tensor_mul(ho[:], go, th[:])
        nc.vector.copy_predicated(h_st[:], mb, ho[:])

        # transposes: h [128, 512] -> hT [128, 4, 128] (both dirs per block)
        for k in range(NKH):
            pt = trpool.tile([128, 128], F16, tag="pt")
            nc.tensor.transpose(pt[:], h_st[:, k * 128:(k + 1) * 128],
                                ident[:])
            if k % 2 == 0:
                nc.vector.tensor_copy(hT[:, k, :], pt[:])
            else:
                nc.scalar.activation(hT[:, k, :], pt[:], AF.Copy)

        if h0T_d is not None:
            for k in range(NKH):
                nc.gpsimd.dma_start(h0T_d[k, tb], hT[:, k, 0:64])
                nc.gpsimd.dma_start(h0T_d[NKH + k, tr], hT[:, k, 64:128])
        if rnnout is not None:
            nc.gpsimd.dma_start(rnnout[:, tb, 0:H], h_st[0:64, :])
            nc.gpsimd.dma_start(rnnout[:, tr, H:2 * H], h_st[64:128, :])

    if h1_out is not None:
        nc.gpsimd.dma_start(h1_out[:], h_st[0:64, :])
        nc.gpsimd.dma_start(c1_out[:], c_st[0:64, :])


@with_exitstack
def build_program(ctx: ExitStack, nc: bass.Bass, add_bias0: bool,
                  add_bias1: bool):
    # ---------------- DRAM parameters ----------------
    def dp(name, shape, dtype, isOutput=False):
        return nc.declare_dram_parameter(name, shape, dtype, isOutput=isOutput)
    xt = dp("xt", [128, NK0, T * B], F16)               # full-x lhsT (replicated)
    w0 = dp("w0", [2, 128, NK0, GSL], F16)              # per-core W0 N-slice
    w1 = dp("w1", [2, 128, NK1, GSL], F16)              # per-core W1 N-slice
    u0 = dp("u0", [2, 128, NKH, G], F16)
    u1 = dp("u1", [2, 128, NKH, G], F16)
    bias0 = dp("bias0", [1, 2, GSL], F16)
    bias1 = dp("bias1", [1, 2, GSL], F16)
    maskc = dp("maskc", [128, T], mybir.dt.uint8)
    ident_in = dp("ident", [128, 128], F16)

    rnnout = dp("rnnout", [B, T, 2 * H], F32, isOutput=True)
    h1_out = dp("h1_out", [B, H], F32, isOutput=True)
    c1_out = dp("c1_out", [B, H], F32, isOutput=True)

    # ---------------- internal DRAM ----------------
    xw0_loc = nc.dram_tensor("xw0_loc", [NCH, 2, TCH, B, GSL], F16)
    xw1_loc = nc.dram_tensor("xw1_loc", [NCH, 2, TCH, B, GSL], F16)
    xw0_g = nc.dram_tensor("xw0_g", [NCH, NCORES, 2, TCH, B, GSL], F16,
                           addr_space="Shared")
    xw1_g = nc.dram_tensor("xw1_g", [NCH, NCORES, 2, TCH, B, GSL], F16,
                           addr_space="Shared")
    h0T_d = nc.dram_tensor("h0T_d", [NK1, T, 128, B], F16)

    tc = ctx.enter_context(tile.TileContext(nc))
    chunks = _chunk_order(NCH)

    # ---------------- persistent SBUF ----------------
    cpool = ctx.enter_context(tc.tile_pool(name="const", bufs=1))
    ident = cpool.tile([128, 128], F16)
    nc.gpsimd.dma_start(ident[:], ident_in[:])
    mask_sb = cpool.tile([128, T], mybir.dt.uint8)
    nc.gpsimd.dma_start(mask_sb[:], maskc[:])
    ones_sb = cpool.tile([1, 128], F16)
    nc.gpsimd.memset(ones_sb[:], 1.0)

    upool = ctx.enter_context(tc.tile_pool(name="upool", bufs=1))
    u0_sb = upool.tile([128, 2, NKH, G], F16, tag="u0")
    nc.gpsimd.dma_start(u0_sb[:], u0[:].rearrange("d p k g -> p d k g"))
    u1_sb = upool.tile([128, 2, NKH, G], F16, tag="u1")
    nc.gpsimd.dma_start(u1_sb[:], u1[:].rearrange("d p k g -> p d k g"))

    # ================= Phase A: GEMM0 + allgather =================
    with tc.tile_pool(name="g0w", bufs=1) as g0w, \
         tc.tile_pool(name="g0l", bufs=4) as g0l:
        w0_sb = g0w.tile([128, 2, NK0, GSL], F16)
        nc.gpsimd.dma_start(w0_sb[:], w0[:].rearrange("d p k g -> p d k g"))
        b0_sb = None
        if add_bias0:
            b0_sb = g0w.tile([1, 2, GSL], F16)
            nc.gpsimd.dma_start(b0_sb[:], bias0[:])

        def load0(m):
            lh = g0l.tile([128, NK0, 128], F16, tag="lh")
            nc.gpsimd.dma_start(lh[:], xt[:, :, m * 128:(m + 1) * 128])
            return lh

        _gemm2(tc, nc, "g0", load0, w0_sb, b0_sb, ones_sb, NK0,
               xw0_loc, xw0_g, chunks)

    # ================= Phase B: layer-0 recurrence =================
    _recurrence(tc, nc, layer=0, u_sb=u0_sb, xw_g=xw0_g, mask_sb=mask_sb,
                ident=ident, h0T_d=h0T_d, rnnout=None, h1_out=None,
                c1_out=None)

    # ================= Phase C: GEMM1 + allgather =================
    with tc.tile_pool(name="g1w", bufs=1) as g1w, \
         tc.tile_pool(name="g1l", bufs=4) as g1l:
        w1_sb = g1w.tile([128, 2, NK1, GSL], F16)
        nc.gpsimd.dma_start(w1_sb[:], w1[:].rearrange("d p k g -> p d k g"))
        b1_sb = None
        if add_bias1:
            b1_sb = g1w.tile([1, 2, GSL], F16)
            nc.gpsimd.dma_start(b1_sb[:], bias1[:])

        def load1(m):
            lh = g1l.tile([128, NK1, 128], F16, tag="lh")
            for k in range(NK1):
                nc.gpsimd.dma_start(
                    lh[:, k, :].rearrange("p (t b) -> p t b", b=B),
                    h0T_d[k, m * 2:(m + 1) * 2, :, :].rearrange("t p b -> p t b"))
            return lh

        _gemm2(tc, nc, "g1", load1, w1_sb, b1_sb, ones_sb, NK1,
               xw1_loc, xw1_g, chunks)

    # ================= Phase D: layer-1 recurrence =================
    _recurrence(tc, nc, layer=1, u_sb=u1_sb, xw_g=xw1_g, mask_sb=mask_sb,
                ident=ident, h0T_d=None, rnnout=rnnout, h1_out=h1_out,
                c1_out=c1_out)


def _gemm2(tc, nc, name, lhs_loader, w_sb, bias_sb, ones_sb, nk,
           xw_loc, xw_g, chunks):
    with tc.tile_pool(name=f"{name}ps", bufs=4, space="PSUM") as ps, \
         tc.tile_pool(name=f"{name}out", bufs=4) as po:
        for c in chunks:
            for mi in range(MT_CH):
                m = c * MT_CH + mi
                lh = lhs_loader(m)
                for d in range(2):
                    zt = ps.tile([128, GSL], F32, tag="z")
                    last = nk - 1
                    for k in range(nk):
                        nc.tensor.matmul(zt[:], lh[:, k, :], w_sb[:, d, k, :],
                                         start=(k == 0),
                                         stop=(k == last and bias_sb is None))
                    if bias_sb is not None:
                        nc.tensor.matmul(zt[:], ones_sb[:], bias_sb[:, d, :],
                                         start=False, stop=True)
                    ot = po.tile([128, GSL], F16, tag="o")
                    if (mi + d) % 2 == 0:
                        nc.vector.tensor_copy(ot[:], zt[:])
                    else:
                        nc.scalar.activation(ot[:], zt[:], AF.Copy)
                    mi2 = (m % MT_CH) * 2
                    nc.gpsimd.dma_start(
                        xw_loc[m // MT_CH, d, mi2:mi2 + 2, :, :]
                        .rearrange("t b n -> (t b) n"),
                        ot[:])
            nc.gpsimd.collective_compute(
                "AllGather", mybir.AluOpType.bypass,
                replica_groups=[list(range(NCORES))],
                ins=[xw_loc[c]],
                outs=[xw_g[c]],
            )


# ======================================================================
# Host side
# ======================================================================

def _ktile(w, nk):
    """[K, Gcols] -> [128, nk, Gcols] fp16 (k-tiled, partition-major)."""
    k, g = w.shape
    assert k == nk * 128
    return np.ascontiguousarray(
        w.reshape(nk, 128, g).transpose(1, 0, 2)).astype(np.float16)


def _prep_w(wf, wb, nk, j):
    """Per-core gate-column slice of both directions: [2, 128, nk, GSL]."""
    sl = slice(j * GSL, (j + 1) * GSL)
    return np.stack([_ktile(np.asarray(wf)[:, sl], nk),
                     _ktile(np.asarray(wb)[:, sl], nk)])


def kernel(x, lengths, W0f, U0f, b0f, W0b, U0b, b0b,
           W1f, U1f, b1f, W1b, U1b, b1b):
    from concourse.bass_utils import run_bass_kernel_spmd

    x = np.asarray(x, dtype=np.float32)
    lengths = np.asarray(lengths)
    add_bias0 = bool(np.any(np.asarray(b0f)) or np.any(np.asarray(b0b)))
    add_bias1 = bool(np.any(np.asarray(b1f)) or np.any(np.asarray(b1b)))

    nc = bacc.Bacc("TRN2", target_bir_lowering=False, debug=False,
                   num_devices=NCORES)
    build_program(nc, add_bias0, add_bias1)
    nc.finalize()

    u0h = np.stack([_ktile(np.asarray(U0f), NKH), _ktile(np.asarray(U0b), NKH)])
    u1h = np.stack([_ktile(np.asarray(U1f), NKH), _ktile(np.asarray(U1b), NKH)])

    # full-x lhsT, t-major columns (col = t*64 + b)
    xth = x.transpose(2, 1, 0).reshape(NK0, 128, T * B)
    xth = np.ascontiguousarray(xth.transpose(1, 0, 2)).astype(np.float16)

    tt = np.arange(T)
    mk = (tt[None, :] < lengths[:, None])
    maskc = np.zeros((128, T), np.uint8)
    maskc[0:64] = mk
    maskc[64:128] = mk[:, ::-1]

    ident = np.eye(128, dtype=np.float16)

    in_maps = []
    for j in range(NCORES):
        sl = slice(j * GSL, (j + 1) * GSL)
        in_maps.append({
            "xt": xth,
            "w0": _prep_w(W0f, W0b, NK0, j),
            "w1": _prep_w(W1f, W1b, NK1, j),
            "u0": u0h, "u1": u1h,
            "bias0": np.stack([np.asarray(b0f)[sl], np.asarray(b0b)[sl]]
                              )[None].astype(np.float16),
            "bias1": np.stack([np.asarray(b1f)[sl], np.asarray(b1b)[sl]]
                              )[None].astype(np.float16),
            "maskc": maskc, "ident": ident,
        })

    res = run_bass_kernel_spmd(nc, in_maps, list(range(NCORES)))
    r0 = res.results[0]
    return (r0["rnnout"].astype(np.float32),
            r0["h1_out"].astype(np.float32),
            r0["c1_out"].astype(np.float32))


if __name__ == "__main__":
    print("kernel module OK")
